# revision 2
# baseline (speedup 1.0000x reference)
"""GAT layer (nn_GATLayer) Trainium2 Bass kernel, 8-core SPMD. v2.

Math: the reference GAT softmax factorizes. scores[n,h,m] =
exp(s_src[n,h]) * exp(s_dst[m,h] + b_attn[h]) * adj_sl[n,m], and the
row-normalization cancels the exp(s_src) factor (EPS=1e-10 is ~1e-11
relative - far below fp32 noise). So with

    e[m,h]  = exp(features[m] @ Wa_dst[h] + b_attn[h])
    ft[m,:] = features[m] @ W_lin.T + b_lin          (128 cols, 2 heads x 64)
    G[m,:]  = [e[m,0]*ft[m,0:64], e[m,1]*ft[m,64:128], e[m,0], e[m,1]]

the whole layer is ONE big matmul  Y = adj_sl @ G  ([8192, 130]) plus
    out[n, h*64+j] = elu(Y[n, h*64+j] / Y[n, 128+h]).

v2 layout: the host pre-transposes and pre-casts each core's adjacency
row-slab to bf16 (0/1/2 are exact in bf16), so the device reads
adjT [8192, 1024] with the contraction index m on partitions:
  - HBM traffic halves (16 MB/core instead of 32 MB fp32)
  - no PE transposes, no PSUM transpose drains, no SWDGE cast
  - each m-band [128, 1024] DMAs as contiguous 2 KB/partition lines
Main loop: for each m-band, 8 matmuls (stationary = adjT block
[128m x 128n], moving = G[mt] [128m x 130]) accumulate into 8 PSUM
banks, one per 128-row destination group. Single bf16 G (no hi/lo
split): measured max rel err vs fp64 oracle is 2.9e-3, dominated by
the bf16 feature matmul, so the lo limb buys nothing.

Sharding: row-shard destination nodes n across 8 cores (1024 rows
each); G / features / weights replicated; no cross-core reduction.
"""

import numpy as np
import ml_dtypes

import concourse.bass as bass
import concourse.mybir as mybir
import concourse.tile as tile
from concourse import bacc
from concourse.bass_utils import run_bass_kernel_spmd

F32 = mybir.dt.float32
BF16 = mybir.dt.bfloat16

N = 8192
IN_DIM = 64
OUT_DIM = 64
HEADS = 2
NCORES = 8
ROWS = N // NCORES          # 1024 destination rows per core
NT = ROWS // 128            # 8 n-tiles per core
MT = N // 128               # 64 m-tiles (full source dim)
C = HEADS * OUT_DIM + HEADS  # 130 columns of G
FT65 = IN_DIM + 1           # features_T plus a ones-row (bias folding)
MG = 4                      # m-bands per DMA group (1 MB transfers)
NGRP = MT // MG             # 16 DMA groups
AF = mybir.ActivationFunctionType


def build_program(reps: int = 1):
    """Trace + compile the SPMD program. reps>1 repeats the whole
    pipeline (for wall-clock slope timing); outputs are overwritten."""
    nc = bacc.Bacc("TRN2", target_bir_lowering=False, debug=False,
                   num_devices=NCORES)

    adjt = nc.dram_tensor("adjt", [N, ROWS], BF16, kind="ExternalInput").ap()
    ft65 = nc.dram_tensor("ft65", [FT65, N], BF16, kind="ExternalInput").ap()
    wcat = nc.dram_tensor("wcat", [FT65, C], BF16, kind="ExternalInput").ap()
    out = nc.dram_tensor("out", [ROWS, HEADS * OUT_DIM], F32,
                         kind="ExternalOutput").ap()

    with tile.TileContext(nc) as tc:
        with tc.tile_pool(name="const", bufs=1) as const, \
             tc.tile_pool(name="gpool", bufs=1) as gpool, \
             tc.tile_pool(name="band_p", bufs=8) as band_p, \
             tc.tile_pool(name="sc_p", bufs=4) as sc_p, \
             tc.tile_pool(name="ep", bufs=3) as ep, \
             tc.tile_pool(name="ps", bufs=8, space="PSUM") as ps_pool:

            ft_sb = const.tile([FT65, N], BF16)
            nc.sync.dma_start(out=ft_sb, in_=ft65)
            wc_sb = const.tile([FT65, C], BF16)
            nc.sync.dma_start(out=wc_sb, in_=wcat)

            for _rep in range(reps):
                # ---- preproc: G[m, :] in bf16, m on partitions ----
                g_sb = gpool.tile([128, MT, C], BF16, name="g_sb")
                for mt in range(MT):
                    psg = ps_pool.tile([128, C], F32, name="psg", tag="ps")
                    nc.tensor.matmul(psg, ft_sb[:, mt * 128:(mt + 1) * 128],
                                     wc_sb, start=True, stop=True)
                    e32 = sc_p.tile([128, HEADS], F32, name="e32")
                    nc.scalar.activation(e32, psg[:, 128:130], AF.Exp)
                    nc.scalar.activation(g_sb[:, mt, 128:130],
                                         psg[:, 128:130], AF.Exp)
                    for h in range(HEADS):
                        nc.vector.tensor_scalar_mul(
                            g_sb[:, mt, h * 64:(h + 1) * 64],
                            psg[:, h * 64:(h + 1) * 64], e32[:, h:h + 1])

                # ---- main: stream adjT m-bands, accumulate 8 PSUM banks ----
                ps_acc = [ps_pool.tile([128, C], F32, name=f"acc{g}", tag="ps")
                          for g in range(NT)]
                for mg in range(NGRP):
                    grp = band_p.tile([128, MG, ROWS], BF16, name="grp")
                    src = adjt[mg * (MG * 128):(mg + 1) * (MG * 128), :]
                    nc.sync.dma_start(
                        out=grp, in_=src.rearrange("(b p) n -> p b n", p=128))
                    for b in range(MG):
                        mt = mg * MG + b
                        for g in range(NT):
                            nc.tensor.matmul(
                                ps_acc[g],
                                grp[:, b, g * 128:(g + 1) * 128],
                                g_sb[:, mt, :],
                                start=(mt == 0), stop=(mt == MT - 1))

                # ---- epilogue: normalize + ELU + store ----
                for g in range(NT):
                    y = ep.tile([128, C], F32, name="y")
                    nc.scalar.copy(y, ps_acc[g])
                    r2 = ep.tile([128, HEADS], F32, name="r2")
                    nc.vector.reciprocal(r2, y[:, 128:130])
                    o1 = ep.tile([128, 128], F32, name="o1")
                    for h in range(HEADS):
                        nc.vector.tensor_scalar_mul(
                            o1[:, h * 64:(h + 1) * 64],
                            y[:, h * 64:(h + 1) * 64], r2[:, h:h + 1])
                    mn = ep.tile([128, 128], F32, name="mn")
                    nc.vector.tensor_scalar_min(mn, o1, 0.0)
                    ex = ep.tile([128, 128], F32, name="ex")
                    nc.scalar.activation(ex, mn, AF.Exp)
                    # elu = (x - min(x,0)) + exp(min(x,0)) - 1
                    nc.vector.tensor_sub(o1, o1, mn)
                    nc.vector.tensor_add(o1, o1, ex)
                    nc.vector.tensor_scalar_add(o1, o1, -1.0)
                    nc.sync.dma_start(out=out[g * 128:(g + 1) * 128, :],
                                      in_=o1)

    nc.compile()
    return nc


def make_in_maps(adj, features, W_attn, b_attn, W_lin, b_lin):
    """Host-side input marshalling: per-core transposed bf16 adjacency
    slabs (+ self-loop diagonal bump), transposed/concatenated small
    operands. adj values are 0/1/2 - exact in bf16."""
    adj = np.asarray(adj, dtype=np.float32)
    features = np.asarray(features, dtype=np.float32)
    W_attn = np.asarray(W_attn, dtype=np.float32)
    b_attn = np.asarray(b_attn, dtype=np.float32)
    W_lin = np.asarray(W_lin, dtype=np.float32)
    b_lin = np.asarray(b_lin, dtype=np.float32)

    BF = ml_dtypes.bfloat16
    ft65 = np.concatenate([features.T.astype(BF),
                           np.ones((1, N), BF)], axis=0)
    ft65 = np.ascontiguousarray(ft65)
    wcat = np.zeros((FT65, C), BF)
    wcat[:IN_DIM, 0:HEADS * OUT_DIM] = W_lin.T.astype(BF)
    wcat[:IN_DIM, HEADS * OUT_DIM:] = W_attn[:, IN_DIM:].T.astype(BF)
    wcat[IN_DIM, 0:HEADS * OUT_DIM] = b_lin.astype(BF)
    wcat[IN_DIM, HEADS * OUT_DIM:] = b_attn.astype(BF)

    adj_bf = adj.astype(BF)  # 0/1 exact
    in_maps = []
    r = np.arange(ROWS)
    for c in range(NCORES):
        slab = np.ascontiguousarray(adj_bf[c * ROWS:(c + 1) * ROWS, :].T)
        slab[c * ROWS + r, r] += np.asarray(1.0, BF)     # self-loops
        in_maps.append({"adjt": slab, "ft65": ft65, "wcat": wcat})
    return in_maps


_CACHED = {}


def _get_program(reps=1):
    if reps not in _CACHED:
        _CACHED[reps] = build_program(reps)
    return _CACHED[reps]


def run_on_device(in_maps, reps=1, **kw):
    nc = _get_program(reps)
    res = run_bass_kernel_spmd(nc, in_maps, core_ids=list(range(NCORES)), **kw)
    return res


def kernel(adj, features, W_attn, b_attn, W_lin, b_lin):
    in_maps = make_in_maps(adj, features, W_attn, b_attn, W_lin, b_lin)
    res = run_on_device(in_maps, reps=1)
    return np.concatenate([res.results[c]["out"] for c in range(NCORES)],
                          axis=0)


# revision 4
# speedup vs baseline: 1.9946x; 1.9946x over previous
"""GAT layer (nn_GATLayer) Trainium2 Bass kernel, 8-core SPMD. v3.

Math: the reference GAT softmax factorizes. scores[n,h,m] =
exp(s_src[n,h]) * exp(s_dst[m,h] + b_attn[h]) * adj_sl[n,m], and the
row-normalization cancels the exp(s_src) factor (EPS=1e-10 is ~1e-11
relative - far below fp32 noise). So with

    e[m,h]  = exp(features[m] @ Wa_dst[h] + b_attn[h])
    ft[m,:] = features[m] @ W_lin.T + b_lin          (128 cols, 2 heads x 64)
    G[m,:]  = [e[m,0]*ft[m,0:64], e[m,1]*ft[m,64:128], e[m,0], e[m,1]]

the whole layer is ONE big matmul  Y = adj_sl @ G  ([8192, 130]) plus
    out[n, h*64+j] = elu(Y[n, h*64+j] / Y[n, 128+h]).

v3 layout: the host pre-transposes each core's adjacency row-slab and
pre-casts it to fp8 E4M3 (0/1/2 are exact), so the device reads
adjT [8192, 1024] fp8 with the contraction index m on partitions:
  - HBM traffic is 1/4 of the fp32 slab (8 MB/core)
  - no PE transposes, no PSUM transpose drains, no cast ops
  - fp8 stationary weights get the 4x fast-weight-load path, so the
    512 accumulation matmuls stream back-to-back at ~N cols/cycle
Main loop: stream 4x 2MB m-band groups; for each of 64 m-bands, 8
matmuls (stationary = adjT block [128m x 128n] fp8, moving = G[mt]
[128m x 130] bf16) accumulate into 8 PSUM banks (two [128,4,512]
bank-aligned accumulator tiles), one bank per 128-row destination
group. Single bf16 G: measured max rel err vs fp64 oracle is 2.9e-3,
dominated by the bf16 feature matmul, so an fp32-ness hi/lo split of
G buys nothing. Preproc and epilogue are batched into a handful of
wide strided ops rather than per-tile chains.

Sharding: row-shard destination nodes n across 8 cores (1024 rows
each); G / features / weights replicated; no cross-core reduction.
"""

import numpy as np
import ml_dtypes

import concourse.bass as bass
import concourse.mybir as mybir
import concourse.tile as tile
from concourse import bacc
from concourse.bass_utils import run_bass_kernel_spmd

F32 = mybir.dt.float32
BF16 = mybir.dt.bfloat16
FP8 = mybir.dt.float8e4
NP_FP8 = ml_dtypes.float8_e4m3
NP_BF16 = ml_dtypes.bfloat16

N = 8192
IN_DIM = 64
OUT_DIM = 64
HEADS = 2
NCORES = 8
ROWS = N // NCORES          # 1024 destination rows per core
NT = ROWS // 128            # 8 n-tiles per core
MT = N // 128               # 64 m-tiles (full source dim)
C = HEADS * OUT_DIM + HEADS  # 130 columns of G
FT65 = IN_DIM + 1           # features_T plus a ones-row (bias folding)
MG = 16                     # m-bands per DMA group (2 MB fp8 transfers)
NGRP = MT // MG             # 4 DMA groups
PMM = 12                    # preproc matmuls drained per PSUM slot (3/bank)
AF = mybir.ActivationFunctionType


def build_program(reps: int = 1):
    """Trace + compile the SPMD program. reps>1 repeats the whole
    pipeline (for wall-clock slope timing); outputs are overwritten."""
    nc = bacc.Bacc("TRN2", target_bir_lowering=False, debug=False,
                   num_devices=NCORES)

    adjt = nc.dram_tensor("adjt", [N, ROWS], FP8, kind="ExternalInput").ap()
    ft65 = nc.dram_tensor("ft65", [FT65, N], BF16, kind="ExternalInput").ap()
    wcat = nc.dram_tensor("wcat", [FT65, C], BF16, kind="ExternalInput").ap()
    out = nc.dram_tensor("out", [ROWS, HEADS * OUT_DIM], F32,
                         kind="ExternalOutput").ap()

    with tile.TileContext(nc) as tc:
        with tc.tile_pool(name="const", bufs=1) as const, \
             tc.tile_pool(name="gpool", bufs=1) as gpool, \
             tc.tile_pool(name="band_p", bufs=2) as band_p, \
             tc.tile_pool(name="ep", bufs=1) as ep, \
             tc.tile_pool(name="ps", bufs=2, space="PSUM") as ps_pool:

            ft_sb = const.tile([FT65, N], BF16)
            nc.sync.dma_start(out=ft_sb, in_=ft65)
            wc_sb = const.tile([FT65, C], BF16)
            nc.sync.dma_start(out=wc_sb, in_=wcat)

            for _rep in range(reps):
                # ---- preproc: ft/e staging in fp32, 12 matmuls per drain ----
                pp = gpool.tile([128, MT, C], F32, name="pp")
                for k0 in range(0, MT, PMM):
                    kn = min(PMM, MT - k0)
                    psg = ps_pool.tile([128, 4, 512], F32, name="psg",
                                       tag="ps")
                    for j in range(kn):
                        mt = k0 + j
                        nc.tensor.matmul(
                            psg[:, j // 3, (j % 3) * C:(j % 3) * C + C],
                            ft_sb[:, mt * 128:(mt + 1) * 128],
                            wc_sb, start=True, stop=True)
                    # drain: one strided copy per run of full 3-mt banks,
                    # plus one for a partial tail bank
                    nfull, rem = divmod(kn, 3)
                    if nfull:
                        src = bass.AP(tensor=psg.tensor, offset=psg.offset,
                                      ap=[list(psg.ap[0]), [512, nfull],
                                          [1, 3 * C]])
                        dst = bass.AP(tensor=pp.tensor,
                                      offset=pp.offset + k0 * C,
                                      ap=[list(pp.ap[0]), [3 * C, nfull],
                                          [1, 3 * C]])
                        nc.vector.tensor_copy(dst, src)
                    if rem:
                        src = bass.AP(tensor=psg.tensor,
                                      offset=psg.offset + nfull * 512,
                                      ap=[list(psg.ap[0]), [1, rem * C]])
                        dst = bass.AP(tensor=pp.tensor,
                                      offset=pp.offset + (k0 + nfull * 3) * C,
                                      ap=[list(pp.ap[0]), [1, rem * C]])
                        nc.vector.tensor_copy(dst, src)
                # e = exp(s_dst + b), written bf16 straight into G
                g_sb = gpool.tile([128, MT, C], BF16, name="g_sb")
                nc.scalar.activation(g_sb[:, :, 128:130], pp[:, :, 128:130],
                                     AF.Exp)
                # G[:, :, h*64:(h+1)*64] = pp * e  (free-step-0 bcast of e)
                for h in range(HEADS):
                    e_rep = bass.AP(tensor=g_sb.tensor,
                                    offset=g_sb.offset + 128 + h,
                                    ap=[list(g_sb.ap[0]), [C, MT],
                                        [0, OUT_DIM]])
                    nc.vector.tensor_mul(g_sb[:, :, h * 64:(h + 1) * 64],
                                         pp[:, :, h * 64:(h + 1) * 64], e_rep)

                # ---- main: stream adjT m-bands, accumulate 8 PSUM banks ----
                acc = [ps_pool.tile([128, 4, 512], F32, name=f"acc{i}",
                                    tag="ps") for i in range(2)]
                for mg in range(NGRP):
                    grp = band_p.tile([128, MG, ROWS], FP8, name="grp")
                    src = adjt[mg * (MG * 128):(mg + 1) * (MG * 128), :]
                    nc.sync.dma_start(
                        out=grp, in_=src.rearrange("(b p) n -> p b n", p=128))
                    for b in range(MG):
                        mt = mg * MG + b
                        for g in range(NT):
                            nc.tensor.matmul(
                                acc[g // 4][:, g % 4, 0:C],
                                grp[:, b, g * 128:(g + 1) * 128],
                                g_sb[:, mt, :],
                                start=(mt == 0), stop=(mt == MT - 1))

                # ---- epilogue: normalize + ELU + store (batched) ----
                y = ep.tile([128, NT, C], F32, name="y")
                nc.scalar.copy(y[:, 0:4, :], acc[0][:, :, 0:C])
                nc.vector.tensor_copy(y[:, 4:8, :], acc[1][:, :, 0:C])
                r2 = ep.tile([128, NT, HEADS], F32, name="r2")
                nc.vector.reciprocal(r2, y[:, :, 128:130])
                o1 = ep.tile([128, NT, 128], F32, name="o1")
                for h in range(HEADS):
                    r_rep = bass.AP(tensor=r2.tensor, offset=r2.offset + h,
                                    ap=[list(r2.ap[0]), [HEADS, NT],
                                        [0, OUT_DIM]])
                    nc.vector.tensor_mul(o1[:, :, h * 64:(h + 1) * 64],
                                         y[:, :, h * 64:(h + 1) * 64], r_rep)
                mn = ep.tile([128, NT, 128], F32, name="mn")
                nc.vector.tensor_scalar_min(mn, o1, 0.0)
                ex = ep.tile([128, NT, 128], F32, name="ex")
                nc.scalar.activation(ex, mn, AF.Exp)
                # elu = (x - min(x,0)) + exp(min(x,0)) - 1
                nc.vector.tensor_sub(o1, o1, mn)
                nc.vector.tensor_add(o1, o1, ex)
                nc.vector.tensor_scalar_add(o1, o1, -1.0)
                nc.sync.dma_start(
                    out=out.rearrange("(g p) c -> p g c", p=128), in_=o1)

    nc.compile()
    return nc


def make_in_maps(adj, features, W_attn, b_attn, W_lin, b_lin):
    """Host-side input marshalling: per-core transposed fp8 adjacency
    slabs (+ self-loop diagonal bump), transposed/concatenated small
    operands. adj values are 0/1/2 - exact in fp8 E4M3."""
    adj = np.asarray(adj, dtype=np.float32)
    features = np.asarray(features, dtype=np.float32)
    W_attn = np.asarray(W_attn, dtype=np.float32)
    b_attn = np.asarray(b_attn, dtype=np.float32)
    W_lin = np.asarray(W_lin, dtype=np.float32)
    b_lin = np.asarray(b_lin, dtype=np.float32)

    ft65 = np.concatenate([features.T.astype(NP_BF16),
                           np.ones((1, N), NP_BF16)], axis=0)
    ft65 = np.ascontiguousarray(ft65)
    wcat = np.zeros((FT65, C), NP_BF16)
    wcat[:IN_DIM, 0:HEADS * OUT_DIM] = W_lin.T.astype(NP_BF16)
    wcat[:IN_DIM, HEADS * OUT_DIM:] = W_attn[:, IN_DIM:].T.astype(NP_BF16)
    wcat[IN_DIM, 0:HEADS * OUT_DIM] = b_lin.astype(NP_BF16)
    wcat[IN_DIM, HEADS * OUT_DIM:] = b_attn.astype(NP_BF16)

    adj_f8 = adj.astype(NP_FP8)  # 0/1 exact
    in_maps = []
    r = np.arange(ROWS)
    for c in range(NCORES):
        slab = np.ascontiguousarray(adj_f8[c * ROWS:(c + 1) * ROWS, :].T)
        slab[c * ROWS + r, r] += np.asarray(1.0, NP_FP8)     # self-loops
        in_maps.append({"adjt": slab, "ft65": ft65, "wcat": wcat})
    return in_maps


_CACHED = {}


def _get_program(reps=1):
    if reps not in _CACHED:
        _CACHED[reps] = build_program(reps)
    return _CACHED[reps]


def run_on_device(in_maps, reps=1, **kw):
    nc = _get_program(reps)
    res = run_bass_kernel_spmd(nc, in_maps, core_ids=list(range(NCORES)), **kw)
    return res


def kernel(adj, features, W_attn, b_attn, W_lin, b_lin):
    in_maps = make_in_maps(adj, features, W_attn, b_attn, W_lin, b_lin)
    res = run_on_device(in_maps, reps=1)
    return np.concatenate([res.results[c]["out"] for c in range(NCORES)],
                          axis=0)


# revision 5
# speedup vs baseline: 8.7168x; 4.3701x over previous
"""GAT layer (nn_GATLayer) Trainium2 Bass kernel, 8-core SPMD. v3.

Math: the reference GAT softmax factorizes. scores[n,h,m] =
exp(s_src[n,h]) * exp(s_dst[m,h] + b_attn[h]) * adj_sl[n,m], and the
row-normalization cancels the exp(s_src) factor (EPS=1e-10 is ~1e-11
relative - far below fp32 noise). So with

    e[m,h]  = exp(features[m] @ Wa_dst[h] + b_attn[h])
    ft[m,:] = features[m] @ W_lin.T + b_lin          (128 cols, 2 heads x 64)
    G[m,:]  = [e[m,0]*ft[m,0:64], e[m,1]*ft[m,64:128], e[m,0], e[m,1]]

the whole layer is ONE big matmul  Y = adj_sl @ G  ([8192, 130]) plus
    out[n, h*64+j] = elu(Y[n, h*64+j] / Y[n, 128+h]).

v3 layout: the host pre-transposes each core's adjacency row-slab and
pre-casts it to fp8 E4M3 (0/1/2 are exact), so the device reads
adjT [8192, 1024] fp8 with the contraction index m on partitions:
  - HBM traffic is 1/4 of the fp32 slab (8 MB/core)
  - no PE transposes, no PSUM transpose drains, no cast ops
  - fp8 stationary weights get the 4x fast-weight-load path, so the
    512 accumulation matmuls stream back-to-back at ~N cols/cycle
Main loop: stream 4x 2MB m-band groups; for each of 64 m-bands, 8
matmuls (stationary = adjT block [128m x 128n] fp8, moving = G[mt]
[128m x 130] bf16) accumulate into 8 PSUM banks (two [128,4,512]
bank-aligned accumulator tiles), one bank per 128-row destination
group. Single bf16 G: measured max rel err vs fp64 oracle is 2.9e-3,
dominated by the bf16 feature matmul, so an fp32-ness hi/lo split of
G buys nothing. Preproc and epilogue are batched into a handful of
wide strided ops rather than per-tile chains.

Sharding: row-shard destination nodes n across 8 cores (1024 rows
each); G / features / weights replicated; no cross-core reduction.
"""

import numpy as np
import ml_dtypes

import concourse.bass as bass
import concourse.mybir as mybir
import concourse.tile as tile
from concourse import bacc
from concourse.bass_utils import run_bass_kernel_spmd

F32 = mybir.dt.float32
BF16 = mybir.dt.bfloat16
FP8 = mybir.dt.float8e4
NP_FP8 = ml_dtypes.float8_e4m3
NP_BF16 = ml_dtypes.bfloat16

N = 8192
IN_DIM = 64
OUT_DIM = 64
HEADS = 2
NCORES = 8
ROWS = N // NCORES          # 1024 destination rows per core
NT = ROWS // 128            # 8 n-tiles per core
MT = N // 128               # 64 m-tiles (full source dim)
C = HEADS * OUT_DIM + HEADS  # 130 columns of G
FT65 = IN_DIM + 1           # features_T plus a ones-row (bias folding)
MG = 32                     # m-bands per DMA group (4 MB fp8 transfers)
NGRP = MT // MG             # 2 DMA groups (double-buffered)
PMM = 12                    # preproc matmuls drained per PSUM slot (3/bank)
AF = mybir.ActivationFunctionType


def build_program(reps: int = 1):
    """Trace + compile the SPMD program. reps>1 repeats the whole
    pipeline (for wall-clock slope timing); outputs are overwritten."""
    nc = bacc.Bacc("TRN2", target_bir_lowering=False, debug=False,
                   num_devices=NCORES)

    adjt = nc.dram_tensor("adjt", [N, ROWS], FP8, kind="ExternalInput").ap()
    ft65 = nc.dram_tensor("ft65", [FT65, N], BF16, kind="ExternalInput").ap()
    wcat = nc.dram_tensor("wcat", [FT65, C], BF16, kind="ExternalInput").ap()
    out = nc.dram_tensor("out", [ROWS, HEADS * OUT_DIM], F32,
                         kind="ExternalOutput").ap()

    with tile.TileContext(nc) as tc:
        with tc.tile_pool(name="const", bufs=1) as const, \
             tc.tile_pool(name="gpool", bufs=1) as gpool, \
             tc.tile_pool(name="band_p", bufs=2) as band_p, \
             tc.tile_pool(name="ep", bufs=1) as ep, \
             tc.tile_pool(name="ps", bufs=2, space="PSUM") as ps_pool:

            ft_sb = const.tile([FT65, N], BF16)
            nc.sync.dma_start(out=ft_sb, in_=ft65)
            wc_sb = const.tile([FT65, C], BF16)
            nc.sync.dma_start(out=wc_sb, in_=wcat)

            for _rep in range(reps):
                # ---- preproc: ft/e staging in fp32, 12 matmuls per drain ----
                pp = gpool.tile([128, MT, C], F32, name="pp")
                for k0 in range(0, MT, PMM):
                    kn = min(PMM, MT - k0)
                    psg = ps_pool.tile([128, 4, 512], F32, name="psg",
                                       tag="ps")
                    for j in range(kn):
                        mt = k0 + j
                        nc.tensor.matmul(
                            psg[:, j // 3, (j % 3) * C:(j % 3) * C + C],
                            ft_sb[:, mt * 128:(mt + 1) * 128],
                            wc_sb, start=True, stop=True)
                    # drain: one strided copy per run of full 3-mt banks,
                    # plus one for a partial tail bank
                    nfull, rem = divmod(kn, 3)
                    if nfull:
                        src = bass.AP(tensor=psg.tensor, offset=psg.offset,
                                      ap=[list(psg.ap[0]), [512, nfull],
                                          [1, 3 * C]])
                        dst = bass.AP(tensor=pp.tensor,
                                      offset=pp.offset + k0 * C,
                                      ap=[list(pp.ap[0]), [3 * C, nfull],
                                          [1, 3 * C]])
                        nc.vector.tensor_copy(dst, src)
                    if rem:
                        src = bass.AP(tensor=psg.tensor,
                                      offset=psg.offset + nfull * 512,
                                      ap=[list(psg.ap[0]), [1, rem * C]])
                        dst = bass.AP(tensor=pp.tensor,
                                      offset=pp.offset + (k0 + nfull * 3) * C,
                                      ap=[list(pp.ap[0]), [1, rem * C]])
                        nc.vector.tensor_copy(dst, src)
                # e = exp(s_dst + b), written bf16 straight into G
                g_sb = gpool.tile([128, MT, C], BF16, name="g_sb")
                nc.scalar.activation(g_sb[:, :, 128:130], pp[:, :, 128:130],
                                     AF.Exp)
                # G[:, :, h*64:(h+1)*64] = pp * e  (free-step-0 bcast of e)
                for h in range(HEADS):
                    e_rep = bass.AP(tensor=g_sb.tensor,
                                    offset=g_sb.offset + 128 + h,
                                    ap=[list(g_sb.ap[0]), [C, MT],
                                        [0, OUT_DIM]])
                    nc.vector.tensor_mul(g_sb[:, :, h * 64:(h + 1) * 64],
                                         pp[:, :, h * 64:(h + 1) * 64], e_rep)

                # ---- main: stream adjT m-bands, accumulate 8 PSUM banks ----
                acc = [ps_pool.tile([128, 4, 512], F32, name=f"acc{i}",
                                    tag="ps") for i in range(2)]
                for mg in range(NGRP):
                    grp = band_p.tile([128, MG, ROWS], FP8, name="grp")
                    src = adjt[mg * (MG * 128):(mg + 1) * (MG * 128), :]
                    nc.sync.dma_start(
                        out=grp, in_=src.rearrange("(b p) n -> p b n", p=128))
                    for b in range(MG):
                        mt = mg * MG + b
                        for g in range(NT):
                            nc.tensor.matmul(
                                acc[g // 4][:, g % 4, 0:C],
                                grp[:, b, g * 128:(g + 1) * 128],
                                g_sb[:, mt, :],
                                start=(mt == 0), stop=(mt == MT - 1))

                # ---- epilogue: normalize + ELU + store (batched) ----
                y = ep.tile([128, NT, C], F32, name="y")
                nc.scalar.copy(y[:, 0:4, :], acc[0][:, :, 0:C])
                nc.vector.tensor_copy(y[:, 4:8, :], acc[1][:, :, 0:C])
                r2 = ep.tile([128, NT, HEADS], F32, name="r2")
                nc.vector.reciprocal(r2, y[:, :, 128:130])
                o1 = ep.tile([128, NT, 128], F32, name="o1")
                for h in range(HEADS):
                    r_rep = bass.AP(tensor=r2.tensor, offset=r2.offset + h,
                                    ap=[list(r2.ap[0]), [HEADS, NT],
                                        [0, OUT_DIM]])
                    nc.vector.tensor_mul(o1[:, :, h * 64:(h + 1) * 64],
                                         y[:, :, h * 64:(h + 1) * 64], r_rep)
                mn = ep.tile([128, NT, 128], F32, name="mn")
                nc.vector.tensor_scalar_min(mn, o1, 0.0)
                ex = ep.tile([128, NT, 128], F32, name="ex")
                nc.scalar.activation(ex, mn, AF.Exp)
                # elu = (x - min(x,0)) + exp(min(x,0)) - 1
                nc.vector.tensor_sub(o1, o1, mn)
                nc.vector.tensor_add(o1, o1, ex)
                nc.vector.tensor_scalar_add(o1, o1, -1.0)
                nc.sync.dma_start(
                    out=out.rearrange("(g p) c -> p g c", p=128), in_=o1)

    nc.compile()
    return nc


def make_in_maps(adj, features, W_attn, b_attn, W_lin, b_lin):
    """Host-side input marshalling: per-core transposed fp8 adjacency
    slabs (+ self-loop diagonal bump), transposed/concatenated small
    operands. adj values are 0/1/2 - exact in fp8 E4M3."""
    adj = np.asarray(adj, dtype=np.float32)
    features = np.asarray(features, dtype=np.float32)
    W_attn = np.asarray(W_attn, dtype=np.float32)
    b_attn = np.asarray(b_attn, dtype=np.float32)
    W_lin = np.asarray(W_lin, dtype=np.float32)
    b_lin = np.asarray(b_lin, dtype=np.float32)

    ft65 = np.concatenate([features.T.astype(NP_BF16),
                           np.ones((1, N), NP_BF16)], axis=0)
    ft65 = np.ascontiguousarray(ft65)
    wcat = np.zeros((FT65, C), NP_BF16)
    wcat[:IN_DIM, 0:HEADS * OUT_DIM] = W_lin.T.astype(NP_BF16)
    wcat[:IN_DIM, HEADS * OUT_DIM:] = W_attn[:, IN_DIM:].T.astype(NP_BF16)
    wcat[IN_DIM, 0:HEADS * OUT_DIM] = b_lin.astype(NP_BF16)
    wcat[IN_DIM, HEADS * OUT_DIM:] = b_attn.astype(NP_BF16)

    adj_f8 = adj.astype(NP_FP8)  # 0/1 exact
    in_maps = []
    r = np.arange(ROWS)
    for c in range(NCORES):
        slab = np.ascontiguousarray(adj_f8[c * ROWS:(c + 1) * ROWS, :].T)
        slab[c * ROWS + r, r] += np.asarray(1.0, NP_FP8)     # self-loops
        in_maps.append({"adjt": slab, "ft65": ft65, "wcat": wcat})
    return in_maps


_CACHED = {}


def _get_program(reps=1):
    if reps not in _CACHED:
        _CACHED[reps] = build_program(reps)
    return _CACHED[reps]


def run_on_device(in_maps, reps=1, **kw):
    nc = _get_program(reps)
    res = run_bass_kernel_spmd(nc, in_maps, core_ids=list(range(NCORES)), **kw)
    return res


def kernel(adj, features, W_attn, b_attn, W_lin, b_lin):
    in_maps = make_in_maps(adj, features, W_attn, b_attn, W_lin, b_lin)
    res = run_on_device(in_maps, reps=1)
    return np.concatenate([res.results[c]["out"] for c in range(NCORES)],
                          axis=0)
